# revision 19
# baseline (speedup 1.0000x reference)
"""Multi-head attention (B=4, S=2048, D=768, H=12) on 8 trn2 NeuronCores.

Sharding: core c handles batch b = c//2 and head-half hh = c%2 (6 heads,
384 features). Each core computes a partial output [2048, 768] (its 6 heads'
contribution through the output projection, un-biased); the host sums the
two partials per batch and adds OW_b plus the V-bias constant
(softmax rows sum to 1, so the V bias contributes OW_w @ VW_b per token).

Speed structure (cost-model-guided; exp throughput is the wall):
  x / weights arrive pre-transposed AND pre-cast to bf16 from the host.
  Q/K projections write fp8e4 tiles (bias fused in the evac) in a "folded"
  layout: [128, 2*S] per m-chunk, col-block 1 = feats 32..63 folded down to
  partitions 0..31 / 64..95 by 2 sbuf DMAs, making the 64-deep score
  contraction 2 k-tiles of 32 partitions -> one fp8 DoubleRow matmul per
  (kc, 512q) at 0.5 cycles/row.
  exp is split tile-alternating between ScalarE (exact exp -> fp8, scale
  fused) and DVE (Schraudolph: fp8 bits = trunc(s*A + B) as a single
  tensor_scalar mult+add writing uint8, bit-viewed as fp8e4; its log-domain
  quantization ~ coincides with fp8's own mantissa rounding).
  AV: stationary-P fp8 DoubleRow over k-chunk pairs: out[128q, 65] +=
  P^T-pair.T @ [V8 | ones]-pair plus a residual pass against [R8 | zeros]
  (R8 = fp8(V - fp8(V)), built by GpSimd) — V stays ~bf16-accurate with
  both operands fp8.  The ones column drops the softmax denominator at
  column 64 of each qtile block; normalize fuses into the psum evac as one
  broadcast tensor_tensor against the reciprocal column.
  attn_sb [128q, qt*128f] xbar-transposes (1 DMA per (qb, m)) into
  attnT [128f, tok] for the output projection.

Scheduling: attention units (h, qb) are software-pipelined.  Unit u's
scores+exp stream is emitted with unit u-1's AV/normalize and the next
m-chunk's projections popped from task queues between score tiles, so the
two exp engines never starve.  PSUM: 3 score slots [128,1024] (6 banks) +
1 AV slot [128,512] (4-qtile passes) + 1 proj slot [128,512] = 8 banks.
"""

from collections import deque

import numpy as np
import ml_dtypes

import concourse.bass as bass
import concourse.mybir as mybir
import concourse.tile as tile
from concourse.bass_utils import run_bass_kernel_spmd

F32 = mybir.dt.float32
BF16 = mybir.dt.bfloat16
FP8 = mybir.dt.float8e4
U8 = mybir.dt.uint8
AF = mybir.ActivationFunctionType
ALU = mybir.AluOpType
DR = mybir.MatmulPerfMode.DoubleRow

B, S, D = 4, 2048, 768
H, HD = 12, 64
N_CORES = 8
HEADS_PER_CORE = 6
FS = HEADS_PER_CORE * HD    # 384
KT16 = S // 128             # 16
QB = 1024
SCALE = 0.125               # 1/sqrt(64)

# Schraudolph exp on DVE: fp8e4 bits = trunc(s_raw * SCH_A + SCH_B).
# SCH_A = 0.125 (score scale) * 8 / ln 2; SCH_B tuned for min final error
# (56.5 - 0.46 under truncating f32->u8 conversion).
SCH_A = 1.4426950408889634
SCH_B = 56.04
ACT_SHARE = 9               # of 16 kc tiles, how many get ScalarE exact exp
# Bresenham-interleaved so ACT and DVE consume the scores psum ring
# concurrently (contiguous ranges serialize the two engines).
ACT_KCS = frozenset(kc for kc in range(16) if (kc * ACT_SHARE) % 16 < ACT_SHARE)


def split_waits(nc, cap=1):
    """walrus rejects instructions carrying >2 sync waits; the TileContext
    final drain is emitted post-lowering and can carry many. Hoist excess
    waits onto preceding same-engine NOPs (1 wait each)."""
    f = nc.m.functions[0]
    for bb in f.blocks:
        insts = list(bb.instructions)
        new = []
        changed = False
        for inst in insts:
            si = inst.sync_info
            if si is not None and si.on_wait is not None and len(si.on_wait) > cap:
                waits = list(si.on_wait)
                keep = waits[-cap:]
                extra = waits[:-cap]
                for j, w in enumerate(extra):
                    nop = mybir.InstNoOp(
                        name=f"{inst.name}-wsplit{j}",
                        engine=inst.engine,
                        ins=[], outs=[],
                        sync_info=mybir.SyncInfo(on_wait=[w], on_update=[]),
                    )
                    new.append(nop)
                    changed = True
                inst.sync_info = mybir.SyncInfo(
                    on_wait=keep, on_update=list(si.on_update or [])
                )
            new.append(inst)
        if changed:
            bb.instructions = new


def build_nc(reps=1, parts="prep,attn,out"):
    nc = bass.Bass()

    x_ext = nc.dram_tensor("xt", [D, S], BF16, kind="ExternalInput")
    wq_ext = nc.dram_tensor("wqt", [D, FS], BF16, kind="ExternalInput")
    wk_ext = nc.dram_tensor("wkt", [D, FS], BF16, kind="ExternalInput")
    wv_ext = nc.dram_tensor("wvt", [D, FS], BF16, kind="ExternalInput")
    wo_ext = nc.dram_tensor("wot", [FS, D], BF16, kind="ExternalInput")
    bq_ext = nc.dram_tensor("bq", [FS], F32, kind="ExternalInput")
    bk_ext = nc.dram_tensor("bk", [FS], F32, kind="ExternalInput")
    out_ext = nc.dram_tensor("out_part", [S, D], F32, kind="ExternalOutput")

    with tile.TileContext(nc) as tc:
      for _rep in range(reps):
        with tc.tile_pool(name="persist", bufs=1) as P:
            xTc = [P.tile([128, S], BF16, name=f"xTc{j}") for j in range(6)]
            wT = {p: P.tile([128, 6 * FS], BF16, name=f"wT{p}") for p in "qkv"}
            woT = P.tile([128, 3 * D], BF16, name="woT")
            QT8 = [P.tile([128, 2 * S], FP8, name=f"QT8{m}") for m in range(3)]
            KT8 = [P.tile([128, 2 * S], FP8, name=f"KT8{m}") for m in range(3)]
            VT = [P.tile([128, S], BF16, name=f"VT{m}") for m in range(3)]
            vtf = [P.tile([128, KT16 * 64], BF16, name=f"vtf{h}")
                   for h in range(HEADS_PER_CORE)]
            v18 = [P.tile([128, KT16 * 65], FP8, name=f"v18h{h}")
                   for h in range(HEADS_PER_CORE)]
            r8 = [P.tile([128, KT16 * 65], FP8, name=f"r8h{h}")
                  for h in range(HEADS_PER_CORE)]
            rbf = [P.tile([128, KT16 * 64], BF16, name=f"rbf{i}")
                   for i in range(2)]
            Pt = [P.tile([128, KT16 * QB], FP8, name=f"Pt{i}")
                  for i in range(2)]
            attn_sb = {(qb, m): P.tile([128, 1024], BF16, name=f"asb{qb}_{m}")
                       for qb in range(2) for m in range(3)}
            attnT = [P.tile([128, S], BF16, name=f"attnT{m}") for m in range(3)]
            qb_sb = P.tile([128, 3], F32, name="qb_sb")
            kb_sb = P.tile([128, 3], F32, name="kb_sb")

            nc.sync.dma_start(qb_sb[:], bq_ext.rearrange("(j p) -> p j", p=128))
            nc.sync.dma_start(kb_sb[:], bk_ext.rearrange("(j p) -> p j", p=128))
            warm = P.tile([128, 1], F32, name="warm")
            nc.vector.memset(warm[:], 0.0)
            nc.scalar.activation(warm[:], warm[:], AF.Exp)

            with (
                tc.tile_pool(name="sp", bufs=1, space="PSUM") as SP,
                tc.tile_pool(name="nw", bufs=3) as NW,
                tc.tile_pool(name="pw", bufs=3) as PW,
            ):
                # ---------- loads (host pre-transposed, pre-cast bf16) ----
                nc.scalar.dma_start(
                    wT["q"][:].rearrange("p (j f) -> p j f", j=6),
                    wq_ext.rearrange("(j p) f -> p j f", p=128))
                nc.scalar.dma_start(
                    wT["k"][:].rearrange("p (j f) -> p j f", j=6),
                    wk_ext.rearrange("(j p) f -> p j f", p=128))
                for j in range(6):
                    nc.sync.dma_start(xTc[j][:], x_ext[j * 128:(j + 1) * 128, :])
                nc.scalar.dma_start(
                    wT["v"][:].rearrange("p (j f) -> p j f", j=6),
                    wv_ext.rearrange("(j p) f -> p j f", p=128))
                nc.scalar.dma_start(
                    woT[:].rearrange("p (j f) -> p j f", j=3),
                    wo_ext.rearrange("(j p) f -> p j f", p=128))

                # ---------- psum slot managers ----------------------------
                sc_idx = [0]

                def sc_tile():
                    i = sc_idx[0] % 3
                    sc_idx[0] += 1
                    return SP.tile([128, QB], F32, tag=f"s{i}", bufs=1,
                                   name=f"ps{i}")

                def av_tile():
                    return SP.tile([128, 512], F32, tag="av", bufs=1,
                                   name="av")

                def pj_tile():
                    return SP.tile([128, 512], F32, tag="pj", bufs=1,
                                   name="pj")

                # ---------- emission helpers ------------------------------
                def proj_chunks(p, m, s4):
                    """projection p, m-chunk, quarter s4 -> two tasks:
                    [alloc+6mm], [evac] (evac decoupled so it never
                    head-blocks the in-order exp engines)"""
                    st = {}

                    def c0():
                        st["ps"] = pj_tile()
                        for kc in range(6):
                            nc.tensor.matmul(
                                st["ps"][:],
                                wT[p][:, kc * FS + m * 128:
                                      kc * FS + (m + 1) * 128],
                                xTc[kc][:, s4 * 512:(s4 + 1) * 512],
                                start=(kc == 0), stop=(kc == 5),
                            )

                    def c1():
                        cols = slice(s4 * 512, (s4 + 1) * 512)
                        if p == "q":
                            nc.vector.tensor_scalar_add(
                                QT8[m][:, cols], st["ps"][:], qb_sb[:, m:m + 1])
                        elif p == "k":
                            nc.vector.tensor_scalar_add(
                                KT8[m][:, cols], st["ps"][:], kb_sb[:, m:m + 1])
                        else:
                            nc.scalar.activation(
                                VT[m][:, cols], st["ps"][:], AF.Copy)

                    return [c0, c1]

                def fold(t8):
                    nc.sync.dma_start(t8[0:32, S:2 * S], t8[32:64, 0:S])
                    nc.sync.dma_start(t8[64:96, S:2 * S], t8[96:128, 0:S])

                def vpath_chunks(h):
                    m, hh = h // 2, h % 2
                    po = hh * 64
                    v1v = v18[h][:].rearrange("p (t c) -> p t c", t=KT16)
                    r1v = r8[h][:].rearrange("p (t c) -> p t c", t=KT16)
                    vtv = vtf[h][:].rearrange("p (t c) -> p t c", t=KT16)
                    rbv = rbf[hh][:].rearrange("p (t c) -> p t c", t=KT16)
                    return [
                        lambda: (nc.sync.dma_start_transpose(
                                    vtf[h][:].rearrange(
                                        "p (t c) -> p t c", t=KT16),
                                    VT[m][po:po + 64, :]),
                                 nc.gpsimd.memset(v18[h][:], 1.0),
                                 nc.gpsimd.memset(r8[h][:], 0.0)),
                        lambda: nc.gpsimd.tensor_copy(v1v[:, :, 0:64], vtv),
                        lambda: nc.gpsimd.tensor_tensor(
                            rbv, vtv, v1v[:, :, 0:64], ALU.subtract),
                        lambda: nc.gpsimd.tensor_copy(r1v[:, :, 0:64], rbv),
                    ]

                def out_proj_task(t):
                    def run():
                        ot = PW.tile([128, D], F32, tag="ot", name="ot")
                        ps = sc_tile()
                        for (lo, hi) in ((0, 512), (512, 768)):
                            for mc in range(3):
                                nc.tensor.matmul(
                                    ps[:, lo:hi],
                                    attnT[mc][:, t * 128:(t + 1) * 128],
                                    woT[:, mc * D + lo:mc * D + hi],
                                    start=(mc == 0), stop=(mc == 2),
                                )
                        if t % 2:
                            nc.scalar.activation(ot[:], ps[:, 0:D], AF.Copy)
                        else:
                            nc.vector.tensor_copy(ot[:], ps[:, 0:D])
                        nc.gpsimd.dma_start(
                            out_ext[t * 128:(t + 1) * 128, :], ot[:])
                    return run

                def av_norm_tasks(h, qb, ptb):
                    """unit (h, qb): AV in two 4-qtile passes + normalize."""
                    m, hh = h // 2, h % 2
                    po = hh * 64
                    ptv = ptb[:].rearrange("p (t c) -> p t c", t=KT16)
                    v1v = v18[h][:].rearrange("p (t c) -> p t c", t=KT16)
                    r1v = r8[h][:].rearrange("p (t c) -> p t c", t=KT16)
                    st = {}
                    tasks = []

                    def mk_av(pa, jlo):
                        def run():
                            if jlo == 0:
                                st["at"] = av_tile()
                            at = st["at"]
                            for j in range(jlo, jlo + 4):
                                for qt in range(4 * pa, 4 * pa + 4):
                                    lh = ptv[:, 2 * j:2 * j + 2,
                                             qt * 128:(qt + 1) * 128]
                                    oc = at[:, (qt % 4) * 128:
                                            (qt % 4) * 128 + 65]
                                    nc.tensor.matmul(
                                        oc, lh, v1v[:, 2 * j:2 * j + 2, :],
                                        start=(j == 0 and qt % 4 == 0),
                                        stop=False, perf_mode=DR,
                                        skip_group_check=True,
                                    )
                                    nc.tensor.matmul(
                                        oc, lh, r1v[:, 2 * j:2 * j + 2, :],
                                        start=False,
                                        stop=(j == 7 and qt % 4 == 3),
                                        perf_mode=DR,
                                        skip_group_check=True,
                                    )
                        return run

                    def mk_norm(pa):
                        def run():
                            at = st["at"]
                            rc = NW.tile([128, 4], F32, tag="rc", name="rc")
                            atv = at[:].rearrange("p (t c) -> p t c", t=4)
                            nc.vector.reciprocal(rc[:], atv[:, :, 64:65])
                            asv = attn_sb[(qb, m)][:].rearrange(
                                "p (t c) -> p t c", t=8)
                            nc.vector.tensor_tensor(
                                asv[:, 4 * pa:4 * pa + 4, po:po + 64],
                                atv[:, :, 0:64],
                                rc[:].unsqueeze(2).broadcast_to([128, 4, 64]),
                                ALU.mult)
                        return run

                    for pa in range(2):
                        tasks.append(mk_av(pa, 0))
                        tasks.append(mk_av(pa, 4))
                        tasks.append(mk_norm(pa))
                    return tasks

                hot = deque()
                cold = deque()  # entries (due_unit, fn)

                def queue_cold(due, fns):
                    cold.extend((due, f) for f in fns)

                def pop_task(every_kc=False, kc=1):
                    # cold (projection) work is paced to odd kc slots so PE
                    # isn't overloaded within a unit; hot work pops anywhere
                    if hot:
                        hot.popleft()()
                    elif cold and (every_kc or len(cold) > 10 or kc % 2 == 1):
                        cold.popleft()[1]()

                # ---------- m = 0 Q/K projections (scores-ring psum) ------
                for p in ("q", "k"):
                    for t2 in range(2):
                        ps = sc_tile()
                        for jh in range(2):
                            for kc in range(6):
                                nc.tensor.matmul(
                                    ps[:, jh * 512:(jh + 1) * 512],
                                    wT[p][:, kc * FS:kc * FS + 128],
                                    xTc[kc][:, t2 * 1024 + jh * 512:
                                             t2 * 1024 + (jh + 1) * 512],
                                    start=(kc == 0), stop=(kc == 5),
                                )
                        cols = slice(t2 * 1024, (t2 + 1) * 1024)
                        if p == "q":
                            nc.vector.tensor_scalar_add(
                                QT8[0][:, cols], ps[:], qb_sb[:, 0:1])
                        else:
                            nc.vector.tensor_scalar_add(
                                KT8[0][:, cols], ps[:], kb_sb[:, 0:1])
                    fold(QT8[0] if p == "q" else KT8[0])

                for s4 in range(4):
                    queue_cold(1, proj_chunks("v", 0, s4))
                queue_cold(1, vpath_chunks(0))
                queue_cold(2, vpath_chunks(1))

                # ---------- pipelined attention units ---------------------
                units = [(m, hh, qb)
                         for m in range(3) for hh in range(2) for qb in range(2)]
                for ui, (m, hh, qb) in enumerate(units):
                    h = 2 * m + hh
                    po = hh * 64
                    # correctness fence: cold work due by this unit must be
                    # EMITTED before this unit's reads (emission order defines
                    # the dependency graph)
                    while cold and cold[0][0] <= ui:
                        cold.popleft()[1]()
                    if hh == 0 and qb == 0 and m + 1 < 3:
                        # dues must be monotone in queue order (fence pops
                        # from the front only)
                        mm_ = m + 1
                        due = 4 * mm_
                        for s4 in range(4):
                            queue_cold(due, proj_chunks("q", mm_, s4))
                        queue_cold(due, [lambda mm_=mm_: fold(QT8[mm_])])
                        for s4 in range(4):
                            queue_cold(due, proj_chunks("k", mm_, s4))
                        queue_cold(due, [lambda mm_=mm_: fold(KT8[mm_])])
                        for s4 in range(4):
                            queue_cold(due + 1, proj_chunks("v", mm_, s4))
                        queue_cold(due + 1, vpath_chunks(2 * mm_))
                        queue_cold(due + 2, vpath_chunks(2 * mm_ + 1))

                    ptb = Pt[ui % 2]
                    ktv = KT8[m][po:po + 32, :].rearrange(
                        "p (t c) -> p t c", t=2)
                    qtv = QT8[m][po:po + 32, :].rearrange(
                        "p (t c) -> p t c", t=2)
                    for kc in range(KT16):
                        sps = sc_tile()
                        for jh in range(2):
                            nc.tensor.matmul(
                                sps[:, jh * 512:(jh + 1) * 512],
                                ktv[:, :, kc * 128:(kc + 1) * 128],
                                qtv[:, :, qb * QB + jh * 512:
                                    qb * QB + (jh + 1) * 512],
                                start=True, stop=True, perf_mode=DR,
                            )
                        pcols = slice(kc * QB, (kc + 1) * QB)
                        if kc in ACT_KCS:
                            nc.scalar.activation(
                                ptb[:, pcols], sps[:], AF.Exp, scale=SCALE)
                        else:
                            nc.vector.tensor_scalar(
                                ptb[:, pcols].bitcast(U8), sps[:],
                                SCH_A, SCH_B, ALU.mult, ALU.add)
                        pop_task(every_kc=(ui == 0), kc=kc)

                    hot.extend(av_norm_tasks(h, qb, ptb))
                    if hh == 1:
                        def trans(m=m, qb=qb):
                            nc.sync.dma_start_transpose(
                                attnT[m][:, qb * QB:(qb + 1) * QB]
                                .rearrange("p (t c) -> p t c", t=8),
                                attn_sb[(qb, m)][:])
                            if m == 2 and qb == 0:
                                queue_cold(12, [out_proj_task(t)
                                                for t in range(8)])
                        hot.append(trans)

                # ---------- drain + tail output projection ----------------
                while hot or cold:
                    if hot:
                        hot.popleft()()
                    else:
                        cold.popleft()[1]()
                for t in range(8, 16):
                    out_proj_task(t)()

    split_waits(nc)
    return nc


_NC_CACHE = None


def _get_nc():
    global _NC_CACHE
    if _NC_CACHE is None:
        _NC_CACHE = build_nc()
    return _NC_CACHE


def make_in_maps(x, QW_w, QW_b, KW_w, KW_b, VW_w, VW_b, OW_w, OW_b):
    f32 = lambda a: np.ascontiguousarray(np.asarray(a), dtype=np.float32)
    bf = lambda a: np.ascontiguousarray(
        np.asarray(np.asarray(a), dtype=np.float32)).astype(ml_dtypes.bfloat16)
    in_maps = []
    for c in range(N_CORES):
        b, hh = c // 2, c % 2
        sl = slice(hh * FS, (hh + 1) * FS)
        in_maps.append({
            "xt": bf(np.asarray(x[b]).T),
            "wqt": bf(np.asarray(QW_w)[sl, :].T),
            "wkt": bf(np.asarray(KW_w)[sl, :].T),
            "wvt": bf(np.asarray(VW_w)[sl, :].T),
            "wot": bf(np.asarray(OW_w)[:, sl].T),
            "bq": f32(QW_b[sl]),
            "bk": f32(KW_b[sl]),
        })
    return in_maps


def kernel(x, QW_w, QW_b, KW_w, KW_b, VW_w, VW_b, OW_w, OW_b):
    nc = _get_nc()
    in_maps = make_in_maps(x, QW_w, QW_b, KW_w, KW_b, VW_w, VW_b, OW_w, OW_b)
    res = run_bass_kernel_spmd(nc, in_maps, list(range(N_CORES)))

    out = np.zeros((B, S, D), dtype=np.float32)
    OW_w = np.asarray(OW_w, dtype=np.float32)
    OW_b = np.asarray(OW_b, dtype=np.float32)
    VW_b = np.asarray(VW_b, dtype=np.float32)
    for c in range(N_CORES):
        b = c // 2
        out[b] += res.results[c]["out_part"]
    for b in range(B):
        # OW bias + V-bias routed through the output projection
        out[b] += OW_b + OW_w @ VW_b
    return out


# revision 59
# speedup vs baseline: 1.1382x; 1.1382x over previous
"""Multi-head attention (B=4, S=2048, D=768, H=12) on 8 trn2 NeuronCores.

Sharding: core c handles batch b = c//2 and head-half hh = c%2 (6 heads,
384 features). Each core computes a partial output [2048, 768] (its 6 heads'
contribution through the output projection, un-biased); the host sums the
two partials per batch and adds OW_b plus the V-bias constant
(softmax rows sum to 1, so the V bias contributes OW_w @ VW_b per token).

Speed structure (cost-model-guided; exp throughput is the wall):
  x / weights arrive pre-transposed AND pre-cast to bf16 from the host.
  Q/K projections write fp8e4 tiles (bias fused in the evac) in a "folded"
  layout: [128, 2*S] per m-chunk, col-block 1 = feats 32..63 folded down to
  partitions 0..31 / 64..95 by 2 sbuf DMAs, making the 64-deep score
  contraction 2 k-tiles of 32 partitions -> one fp8 DoubleRow matmul per
  (kc, 512q) at 0.5 cycles/row.
  exp is split tile-alternating between ScalarE (exact exp -> fp8, scale
  fused) and DVE (Schraudolph: fp8 bits = trunc(s*A + B) as a single
  tensor_scalar mult+add writing uint8, bit-viewed as fp8e4; its log-domain
  quantization ~ coincides with fp8's own mantissa rounding).
  AV: stationary-P fp8 DoubleRow over k-chunk pairs: out[128q, 65] +=
  P^T-pair.T @ [V8 | ones]-pair plus a residual pass against [R8 | zeros]
  (R8 = fp8(V - fp8(V)), built by GpSimd) — V stays ~bf16-accurate with
  both operands fp8.  The ones column drops the softmax denominator at
  column 64 of each qtile block; normalize fuses into the psum evac as one
  broadcast tensor_tensor against the reciprocal column.
  attn_sb [128q, qt*128f] xbar-transposes (1 DMA per (qb, m)) into
  attnT [128f, tok] for the output projection.

Scheduling: attention units (h, qb) are software-pipelined.  Unit u's
scores+exp stream is emitted with unit u-1's AV/normalize and the next
m-chunk's projections popped from task queues between score tiles, so the
two exp engines never starve.  PSUM: 3 score slots [128,1024] (6 banks) +
1 AV slot [128,512] (4-qtile passes) + 1 proj slot [128,512] = 8 banks.
"""

from collections import deque

import numpy as np
import ml_dtypes

import concourse.bass as bass
import concourse.mybir as mybir
import concourse.tile as tile
from concourse.bass_utils import run_bass_kernel_spmd

F32 = mybir.dt.float32
BF16 = mybir.dt.bfloat16
FP8 = mybir.dt.float8e4
U8 = mybir.dt.uint8
AF = mybir.ActivationFunctionType
ALU = mybir.AluOpType
DR = mybir.MatmulPerfMode.DoubleRow

B, S, D = 4, 2048, 768
H, HD = 12, 64
N_CORES = 8
HEADS_PER_CORE = 6
FS = HEADS_PER_CORE * HD    # 384
KT16 = S // 128             # 16
QB = 1024
SCALE = 0.125               # 1/sqrt(64)

# Schraudolph exp on DVE: fp8e4 bits = round(s_raw * SCH_A + SCH_B).
# SCH_A = 0.125 (score scale) * 8 / ln 2; SCH_B tuned for min final error
# (56 - 0.46; hardware f32->u8 conversion rounds to nearest).
SCH_A = 1.4426950408889634
SCH_B = 55.54
ACT_SHARE = 9               # of 16 kc tiles, how many get ScalarE exact exp
# Bresenham-interleaved so ACT and DVE consume the scores psum ring
# concurrently (contiguous ranges serialize the two engines).
ACT_KCS = frozenset(kc for kc in range(16) if (kc * ACT_SHARE) % 16 < ACT_SHARE)


def split_waits(nc, cap=1):
    """walrus rejects instructions carrying >2 sync waits; the TileContext
    final drain is emitted post-lowering and can carry many. Hoist excess
    waits onto preceding same-engine NOPs (1 wait each)."""
    f = nc.m.functions[0]
    for bb in f.blocks:
        insts = list(bb.instructions)
        new = []
        changed = False
        for inst in insts:
            si = inst.sync_info
            if si is not None and si.on_wait is not None and len(si.on_wait) > cap:
                waits = list(si.on_wait)
                keep = waits[-cap:]
                extra = waits[:-cap]
                for j, w in enumerate(extra):
                    nop = mybir.InstNoOp(
                        name=f"{inst.name}-wsplit{j}",
                        engine=inst.engine,
                        ins=[], outs=[],
                        sync_info=mybir.SyncInfo(on_wait=[w], on_update=[]),
                    )
                    new.append(nop)
                    changed = True
                inst.sync_info = mybir.SyncInfo(
                    on_wait=keep, on_update=list(si.on_update or [])
                )
            new.append(inst)
        if changed:
            bb.instructions = new


def build_nc(reps=1, parts="prep,attn,out"):
    nc = bass.Bass()

    x_ext = nc.dram_tensor("xt", [D, S], BF16, kind="ExternalInput")
    wq_ext = nc.dram_tensor("wqt", [D, FS], BF16, kind="ExternalInput")
    wk_ext = nc.dram_tensor("wkt", [D, FS], BF16, kind="ExternalInput")
    wv_ext = nc.dram_tensor("wvt", [D, FS], BF16, kind="ExternalInput")
    wo_ext = nc.dram_tensor("wot", [FS, D], BF16, kind="ExternalInput")
    bq_ext = nc.dram_tensor("bq", [FS], F32, kind="ExternalInput")
    bk_ext = nc.dram_tensor("bk", [FS], F32, kind="ExternalInput")
    out_ext = nc.dram_tensor("out_part", [S, D], F32, kind="ExternalOutput")

    with tile.TileContext(nc) as tc:
      for _rep in range(reps):
        with tc.tile_pool(name="persist", bufs=1) as P:
            xTc = [P.tile([128, S], BF16, name=f"xTc{j}") for j in range(6)]
            wT = {p: P.tile([128, 6 * FS], BF16, name=f"wT{p}") for p in "qkv"}
            woT = P.tile([128, 3 * D], BF16, name="woT")
            QT8 = [P.tile([128, 2 * S], FP8, name=f"QT8{m}") for m in range(3)]
            KT8 = [P.tile([128, 2 * S], FP8, name=f"KT8{m}") for m in range(3)]
            VT = [P.tile([128, S], BF16, name=f"VT{m}") for m in range(3)]
            vtf = [P.tile([128, KT16 * 64], BF16, name=f"vtf{h}")
                   for h in range(HEADS_PER_CORE)]
            v18 = [P.tile([128, KT16 * 65], FP8, name=f"v18h{h}")
                   for h in range(HEADS_PER_CORE)]
            r8 = [P.tile([128, KT16 * 65], FP8, name=f"r8h{h}")
                  for h in range(HEADS_PER_CORE)]
            rbf = [P.tile([128, KT16 * 64], BF16, name=f"rbf{i}")
                   for i in range(2)]
            Pt = [P.tile([128, KT16 * QB], FP8, name=f"Pt{i}")
                  for i in range(2)]
            attn_sb = {(qb, m): P.tile([128, 1024], BF16, name=f"asb{qb}_{m}")
                       for qb in range(2) for m in range(3)}
            attnT = [P.tile([128, S], BF16, name=f"attnT{m}") for m in range(3)]
            qb_sb = P.tile([128, 3], F32, name="qb_sb")
            kb_sb = P.tile([128, 3], F32, name="kb_sb")

            nc.sync.dma_start(qb_sb[:], bq_ext.rearrange("(j p) -> p j", p=128))
            nc.sync.dma_start(kb_sb[:], bk_ext.rearrange("(j p) -> p j", p=128))
            warm = P.tile([128, 1], F32, name="warm")
            nc.vector.memset(warm[:], 0.0)
            nc.scalar.activation(warm[:], warm[:], AF.Exp)

            with (
                tc.tile_pool(name="sp", bufs=1, space="PSUM") as SP,
                tc.tile_pool(name="nw", bufs=3) as NW,
                tc.tile_pool(name="pw", bufs=3) as PW,
            ):
                # ---------- loads (host pre-transposed, pre-cast bf16;
                # only what gates unit 0 loads up front — token-half 1 of
                # x plus wv/wo ride the task queue so the Q/K fold DMAs
                # aren't stuck behind them on the serial DMA engines) ------
                nc.scalar.dma_start(
                    wT["q"][:].rearrange("p (j f) -> p j f", j=6),
                    wq_ext.rearrange("(j p) f -> p j f", p=128))
                nc.scalar.dma_start(
                    wT["k"][:].rearrange("p (j f) -> p j f", j=6),
                    wk_ext.rearrange("(j p) f -> p j f", p=128))
                for j in range(6):
                    nc.sync.dma_start(
                        xTc[j][:, 0:QB], x_ext[j * 128:(j + 1) * 128, 0:QB])
                for j in range(6):
                    nc.sync.dma_start(
                        xTc[j][:, QB:S], x_ext[j * 128:(j + 1) * 128, QB:S])

                def load_wv():
                    nc.scalar.dma_start(
                        wT["v"][:].rearrange("p (j f) -> p j f", j=6),
                        wv_ext.rearrange("(j p) f -> p j f", p=128))

                def load_wo():
                    nc.scalar.dma_start(
                        woT[:].rearrange("p (j f) -> p j f", j=3),
                        wo_ext.rearrange("(j p) f -> p j f", p=128))

                # ---------- psum slot managers ----------------------------
                sc_idx = [0]

                def sc_tile():
                    i = sc_idx[0] % 3
                    sc_idx[0] += 1
                    return SP.tile([128, QB], F32, tag=f"s{i}", bufs=1,
                                   name=f"ps{i}")

                def av_tile():
                    return SP.tile([128, 512], F32, tag="av", bufs=1,
                                   name="av")

                def pj_tile():
                    return SP.tile([128, 512], F32, tag="pj", bufs=1,
                                   name="pj")

                # ---------- emission helpers ------------------------------
                def proj_chunks(p, m, s4, tile_fn=None):
                    """projection p, m-chunk, quarter s4 -> two tasks:
                    [alloc+6mm], [evac] (evac decoupled so it never
                    head-blocks the in-order exp engines)"""
                    st = {}

                    def c0():
                        st["ps"] = (tile_fn or pj_tile)()
                        for kc in range(6):
                            nc.tensor.matmul(
                                st["ps"][:, 0:512],
                                wT[p][:, kc * FS + m * 128:
                                      kc * FS + (m + 1) * 128],
                                xTc[kc][:, s4 * 512:(s4 + 1) * 512],
                                start=(kc == 0), stop=(kc == 5),
                            )

                    def c1():
                        cols = slice(s4 * 512, (s4 + 1) * 512)
                        ps = st["ps"][:, 0:512]
                        if p == "q":
                            nc.vector.tensor_scalar_add(
                                QT8[m][:, cols], ps, qb_sb[:, m:m + 1])
                        elif p == "k":
                            nc.vector.tensor_scalar_add(
                                KT8[m][:, cols], ps, kb_sb[:, m:m + 1])
                        else:
                            nc.scalar.activation(
                                VT[m][:, cols], ps, AF.Copy)

                    return [c0, c1]

                def fold(t8, half=None):
                    c0, c1 = (0, S) if half is None else \
                        (half * QB, (half + 1) * QB)
                    nc.sync.dma_start(t8[0:32, S + c0:S + c1],
                                      t8[32:64, c0:c1])
                    nc.sync.dma_start(t8[64:96, S + c0:S + c1],
                                      t8[96:128, c0:c1])

                def vpath_chunks(h, eng=None):
                    # head 0 gates the first AV batch: run its quantize
                    # chain on DVE (2x SBUF mode) instead of the slower Pool
                    m, hh = h // 2, h % 2
                    po = hh * 64
                    e = eng or nc.gpsimd
                    v1v = v18[h][:].rearrange("p (t c) -> p t c", t=KT16)
                    r1v = r8[h][:].rearrange("p (t c) -> p t c", t=KT16)
                    vtv = vtf[h][:].rearrange("p (t c) -> p t c", t=KT16)
                    rbv = rbf[hh][:].rearrange("p (t c) -> p t c", t=KT16)
                    return [
                        lambda: (nc.sync.dma_start_transpose(
                                    vtf[h][:].rearrange(
                                        "p (t c) -> p t c", t=KT16),
                                    VT[m][po:po + 64, :]),
                                 nc.gpsimd.memset(v18[h][:], 1.0),
                                 nc.gpsimd.memset(r8[h][:], 0.0)),
                        lambda: e.tensor_copy(v1v[:, :, 0:64], vtv),
                        lambda: e.tensor_tensor(
                            rbv, vtv, v1v[:, :, 0:64], ALU.subtract),
                        lambda: e.tensor_copy(r1v[:, :, 0:64], rbv),
                    ]

                def out_proj_task(t):
                    def run():
                        ot = PW.tile([128, D], F32, tag="ot", name="ot", bufs=6)
                        ps = sc_tile()
                        for (lo, hi) in ((0, 512), (512, 768)):
                            for mc in range(3):
                                nc.tensor.matmul(
                                    ps[:, lo:hi],
                                    attnT[mc][:, t * 128:(t + 1) * 128],
                                    woT[:, mc * D + lo:mc * D + hi],
                                    start=(mc == 0), stop=(mc == 2),
                                )
                        if t % 2:
                            nc.scalar.activation(ot[:], ps[:, 0:D], AF.Copy)
                        else:
                            nc.vector.tensor_copy(ot[:], ps[:, 0:D])
                        nc.gpsimd.dma_start(
                            out_ext[t * 128:(t + 1) * 128, :], ot[:])
                    return run

                def av_norm_tasks(h, qb, ptb):
                    """unit (h, qb): AV in two 4-qtile passes + normalize."""
                    m, hh = h // 2, h % 2
                    po = hh * 64
                    ptv = ptb[:].rearrange("p (t c) -> p t c", t=KT16)
                    v1v = v18[h][:].rearrange("p (t c) -> p t c", t=KT16)
                    r1v = r8[h][:].rearrange("p (t c) -> p t c", t=KT16)
                    st = {}
                    tasks = []

                    def mk_av(pa, jlo):
                        def run():
                            if jlo == 0:
                                st["at"] = av_tile()
                            at = st["at"]
                            for j in range(jlo, jlo + 4):
                                for qt in range(4 * pa, 4 * pa + 4):
                                    lh = ptv[:, 2 * j:2 * j + 2,
                                             qt * 128:(qt + 1) * 128]
                                    oc = at[:, (qt % 4) * 128:
                                            (qt % 4) * 128 + 65]
                                    nc.tensor.matmul(
                                        oc, lh, v1v[:, 2 * j:2 * j + 2, :],
                                        start=(j == 0 and qt % 4 == 0),
                                        stop=False, perf_mode=DR,
                                        skip_group_check=True,
                                    )
                                    nc.tensor.matmul(
                                        oc, lh, r1v[:, 2 * j:2 * j + 2, :],
                                        start=False,
                                        stop=(j == 7 and qt % 4 == 3),
                                        perf_mode=DR,
                                        skip_group_check=True,
                                    )
                        return run

                    def mk_norm(pa):
                        def run():
                            at = st["at"]
                            rc = NW.tile([128, 4], F32, tag="rc", name="rc")
                            atv = at[:].rearrange("p (t c) -> p t c", t=4)
                            nc.vector.reciprocal(rc[:], atv[:, :, 64:65])
                            asv = attn_sb[(qb, m)][:].rearrange(
                                "p (t c) -> p t c", t=8)
                            nc.vector.tensor_tensor(
                                asv[:, 4 * pa:4 * pa + 4, po:po + 64],
                                atv[:, :, 0:64],
                                rc[:].unsqueeze(2).broadcast_to([128, 4, 64]),
                                ALU.mult)
                        return run

                    for pa in range(2):
                        tasks.append(mk_av(pa, 0))
                        tasks.append(mk_av(pa, 4))
                        tasks.append(mk_norm(pa))
                    return tasks

                hot = deque()
                cold = deque()  # entries (due_unit, fn)

                def queue_cold(due, fns):
                    cold.extend((due, f) for f in fns)

                def pop_task(every_kc=False, kc=1):
                    # cold (projection) work is paced to odd kc slots so PE
                    # isn't overloaded within a unit; hot work pops anywhere
                    if hot:
                        hot.popleft()()
                    elif cold and (every_kc or len(cold) > 10 or kc % 2 == 1):
                        cold.popleft()[1]()
                        if cold and len(cold) > 24:
                            cold.popleft()[1]()

                # ---------- m = 0 Q/K projections: token-half 0 inline via
                # the pj/av slots (the scores ring stays clean so unit 0
                # pipelines from the first tile); half 1 + V + v-paths ride
                # the task queue.  Units 0/1 run non-DoubleRow scores off
                # the stage region, so the fold DMAs leave the startup
                # critical path (folds are due by unit 2). ----------------
                # K must cover ALL key tokens before unit 0's kc loop
                # (kc spans the full sequence); Q only needs its qb half.
                for s4, fn in ((0, pj_tile), (1, av_tile)):
                    for c in proj_chunks("q", 0, s4, tile_fn=fn):
                        c()
                for s4, fn in ((0, sc_tile), (1, sc_tile)):
                    for c in proj_chunks("k", 0, s4, tile_fn=fn):
                        c()
                fold(QT8[0], half=0)
                fold(KT8[0], half=0)
                for s4, fn in ((2, sc_tile), (3, pj_tile)):
                    for c in proj_chunks("k", 0, s4, tile_fn=fn):
                        c()
                fold(KT8[0], half=1)
                queue_cold(1, [load_wv])
                queue_cold(1, proj_chunks("q", 0, 2, tile_fn=pj_tile)
                           + proj_chunks("q", 0, 3, tile_fn=av_tile)
                           + [lambda: fold(QT8[0], half=1)])
                for s4 in range(4):
                    queue_cold(1, proj_chunks("v", 0, s4))
                queue_cold(1, vpath_chunks(0, eng=nc.vector))
                queue_cold(2, vpath_chunks(1) + [load_wo])

                # ---------- pipelined attention units ---------------------
                units = [(m, hh, qb)
                         for m in range(3) for hh in range(2) for qb in range(2)]
                for ui, (m, hh, qb) in enumerate(units):
                    h = 2 * m + hh
                    po = hh * 64
                    # correctness fence: cold work due by this unit must be
                    # EMITTED before this unit's reads (emission order defines
                    # the dependency graph)
                    while cold and cold[0][0] <= ui:
                        cold.popleft()[1]()
                    if hh == 0 and qb == 0 and m + 1 < 3:
                        # dues must be monotone in queue order (fence pops
                        # from the front only); V/v-path spill into the next
                        # m-window where PE has spare capacity
                        mm_ = m + 1
                        due = 4 * mm_
                        for s4 in range(4):
                            queue_cold(due, proj_chunks("q", mm_, s4))
                        queue_cold(due, [lambda mm_=mm_: fold(QT8[mm_])])
                        for s4 in range(4):
                            queue_cold(due, proj_chunks("k", mm_, s4))
                        queue_cold(due, [lambda mm_=mm_: fold(KT8[mm_])])
                        for s4 in range(4):
                            queue_cold(due + 1, proj_chunks("v", mm_, s4))
                        queue_cold(due + 1, vpath_chunks(2 * mm_))
                        queue_cold(due + 2, vpath_chunks(2 * mm_ + 1))

                    ptb = Pt[ui % 2]
                    ktv = KT8[m][po:po + 32, :].rearrange(
                        "p (t c) -> p t c", t=2)
                    qtv = QT8[m][po:po + 32, :].rearrange(
                        "p (t c) -> p t c", t=2)
                    n_act = ACT_SHARE + (ui % 2)
                    act_kcs = frozenset(
                        kc for kc in range(16) if (kc * n_act) % 16 < n_act)
                    for kc in range(KT16):
                        sps = sc_tile()
                        for jh in range(2):
                            nc.tensor.matmul(
                                sps[:, jh * 512:(jh + 1) * 512],
                                ktv[:, :, kc * 128:(kc + 1) * 128],
                                qtv[:, :, qb * QB + jh * 512:
                                    qb * QB + (jh + 1) * 512],
                                start=True, stop=True, perf_mode=DR,
                            )
                        pcols = slice(kc * QB, (kc + 1) * QB)
                        if kc in act_kcs:
                            nc.scalar.activation(
                                ptb[:, pcols], sps[:], AF.Exp, scale=SCALE)
                        else:
                            nc.vector.tensor_scalar(
                                ptb[:, pcols].bitcast(U8), sps[:],
                                SCH_A, SCH_B, ALU.mult, ALU.add)
                        pop_task(every_kc=(ui == 0), kc=kc)

                    hot.extend(av_norm_tasks(h, qb, ptb))
                    if hh == 1:
                        def trans(m=m, qb=qb):
                            nc.sync.dma_start_transpose(
                                attnT[m][:, qb * QB:(qb + 1) * QB]
                                .rearrange("p (t c) -> p t c", t=8),
                                attn_sb[(qb, m)][:])
                            if m == 2 and qb == 0:
                                queue_cold(12, [out_proj_task(t)
                                                for t in range(8)])
                            elif m == 2 and qb == 1:
                                queue_cold(12, [out_proj_task(t)
                                                for t in range(8, 16)])
                        hot.append(trans)

                # ---------- drain (tail out-projections ride the queues) --
                while hot or cold:
                    if hot:
                        hot.popleft()()
                    else:
                        cold.popleft()[1]()

    split_waits(nc)
    return nc


_NC_CACHE = None


def _get_nc():
    global _NC_CACHE
    if _NC_CACHE is None:
        _NC_CACHE = build_nc()
    return _NC_CACHE


def make_in_maps(x, QW_w, QW_b, KW_w, KW_b, VW_w, VW_b, OW_w, OW_b):
    f32 = lambda a: np.ascontiguousarray(np.asarray(a), dtype=np.float32)
    bf = lambda a: np.ascontiguousarray(
        np.asarray(np.asarray(a), dtype=np.float32)).astype(ml_dtypes.bfloat16)
    in_maps = []
    for c in range(N_CORES):
        b, hh = c // 2, c % 2
        sl = slice(hh * FS, (hh + 1) * FS)
        in_maps.append({
            "xt": bf(np.asarray(x[b]).T),
            "wqt": bf(np.asarray(QW_w)[sl, :].T),
            "wkt": bf(np.asarray(KW_w)[sl, :].T),
            "wvt": bf(np.asarray(VW_w)[sl, :].T),
            "wot": bf(np.asarray(OW_w)[:, sl].T),
            "bq": f32(QW_b[sl]),
            "bk": f32(KW_b[sl]),
        })
    return in_maps


def kernel(x, QW_w, QW_b, KW_w, KW_b, VW_w, VW_b, OW_w, OW_b):
    nc = _get_nc()
    in_maps = make_in_maps(x, QW_w, QW_b, KW_w, KW_b, VW_w, VW_b, OW_w, OW_b)
    res = run_bass_kernel_spmd(nc, in_maps, list(range(N_CORES)))

    out = np.zeros((B, S, D), dtype=np.float32)
    OW_w = np.asarray(OW_w, dtype=np.float32)
    OW_b = np.asarray(OW_b, dtype=np.float32)
    VW_b = np.asarray(VW_b, dtype=np.float32)
    for c in range(N_CORES):
        b = c // 2
        out[b] += res.results[c]["out_part"]
    for b in range(B):
        # OW bias + V-bias routed through the output projection
        out[b] += OW_b + OW_w @ VW_b
    return out
